# revision 15
# baseline (speedup 1.0000x reference)
"""Trainium2 Bass/Tile kernel for the HairBundle SDE drift+diffusion.

Contract: kernel(t, x) takes the FULL inputs (t: [1] f32, x: [8_000_000, 5]
f32) and returns the full (drift, diffusion) pair, matching reference().

Strategy
--------
Trivially data-parallel over the sample-path axis: 8 NeuronCores, each core
takes 1M rows, padded to 128*7813 rows and laid out [128 partitions, 7813
rows/partition, 5 components] so every DMA is dense (contiguous 5*F floats
per partition).  Per 128xFx5 tile the drift is evaluated with 9 VectorE
streams (tensor_tensor / scalar_tensor_tensor) + 9 ScalarE(ACT) streams
(sigmoid + affine Identity ops), reading/writing the interleaved component
planes through stride-5 access patterns.  The diffusion output is a
constant broadcast and is produced host-side for free.

Math (constants folded from the reference):
    d  = h - a;  po = sigmoid(4 d)
    dh = 0.375*(2*(-1.8 h + a) + po) + force         (ACT bias = force)
    da = -0.06*((5/3) dh + (h + 0.75 a - 0.525 m)) + (0.1*force - 0.035)
    dv = (v - 1)*(-c*po - k) - k   for (v,c,k) in
         (m,1.2,0.8), (g,0.7,0.5), (t,0.3,0.4)
    force = 0.5*sin(2*pi*t)
"""

import numpy as np

_B = 8_000_000
_NCORES = 8
_RPC = _B // _NCORES            # rows per core = 1_000_000
_P = 128
_Q = -(-_RPC // _P)             # 7813 rows per partition (padded)
_PADROWS = _P * _Q - _RPC       # 64 junk rows at the tail of each shard
_F = 768                        # rows-per-partition per SBUF tile
_DSIG = np.array([0.05, 0.02, 0.0, 0.0, 0.0], dtype=np.float32)

_CACHE = {}


def _build_nc(q, f):
    """Build the per-core Bass program for [128, q, 5] in/out, tile width f."""
    import concourse.bacc as bacc
    import concourse.mybir as mybir
    import concourse.tile as tile

    f32 = mybir.dt.float32
    Act = mybir.ActivationFunctionType
    Op = mybir.AluOpType

    nc = bacc.Bacc("TRN2", debug=False)
    x_d = nc.dram_tensor("x", [_P, q, 5], f32, kind="ExternalInput").ap()
    c_d = nc.dram_tensor("consts", [_P, 5], f32, kind="ExternalInput").ap()
    o_d = nc.dram_tensor("drift", [_P, q, 5], f32, kind="ExternalOutput").ap()

    ntiles = -(-q // f)

    with tile.TileContext(nc) as tc:
        with (
            tc.tile_pool(name="io", bufs=3) as io_pool,
            tc.tile_pool(name="tmp", bufs=2) as tmp_pool,
            tc.tile_pool(name="cst", bufs=1) as cst_pool,
        ):
            consts = cst_pool.tile([_P, 5], f32, name="consts_sb")
            nc.sync.dma_start(consts[:, :], c_d[:, :])
            force_b = consts[:, 0:1]
            cprime_b = consts[:, 1:2]
            km_b = consts[:, 2:3]   # -0.8
            kg_b = consts[:, 3:4]   # -0.5
            kt_b = consts[:, 4:5]   # -0.4

            for ti in range(ntiles):
                f0 = ti * f
                fw = min(f, q - f0)

                X = io_pool.tile([_P, f, 5], f32, tag="X", name="X")
                nc.sync.dma_start(X[:, :fw, :], x_d[:, f0 : f0 + fw, :])
                D = io_pool.tile([_P, f, 5], f32, tag="D", name="D")

                h = X[:, :fw, 0]
                a = X[:, :fw, 1]
                m = X[:, :fw, 2]
                g = X[:, :fw, 3]
                t_ = X[:, :fw, 4]
                dh = D[:, :fw, 0]
                da = D[:, :fw, 1]
                dm = D[:, :fw, 2]
                dg = D[:, :fw, 3]
                dt = D[:, :fw, 4]

                def T(nm, bufs=2):
                    # bufs=1 for temps consumed on the same engine that wrote
                    # them (in-order engines make the WAR free); bufs=2 for
                    # cross-engine temps so tiles can pipeline
                    return tmp_pool.tile([_P, f], f32, tag=nm, name=nm, bufs=bufs)[
                        :, :fw
                    ]

                d = T("d")
                po = T("po")
                u1 = T("u1", 1)
                u2 = T("u2")
                z1 = T("z1", 1)
                z2 = T("z2", 1)
                zp = T("zp")
                qm = T("qm")
                qg = T("qg")
                qt = T("qt")
                m1 = T("m1", 1)
                g1 = T("g1", 1)
                t1 = T("t1", 1)
                pm = T("pm")
                pg = T("pg")
                pt = T("pt")

                # d = h - a ; po = sigmoid(4 d)
                nc.vector.tensor_tensor(d, h, a, Op.subtract)
                nc.scalar.activation(po, d, Act.Sigmoid, scale=4.0)

                # dh = 0.375*(2*(a - 1.8 h) + po) + force
                nc.vector.scalar_tensor_tensor(u1, h, -1.8, a, Op.mult, Op.add)
                nc.vector.scalar_tensor_tensor(u2, u1, 2.0, po, Op.mult, Op.add)
                nc.scalar.activation(dh, u2, Act.Identity, bias=force_b, scale=0.375)

                # da = 0.0375*(2h - 3.2a - po + 0.84m) - 0.035
                nc.vector.scalar_tensor_tensor(z1, a, -1.6, h, Op.mult, Op.add)
                nc.vector.scalar_tensor_tensor(z2, z1, 2.0, po, Op.mult, Op.subtract)
                nc.vector.scalar_tensor_tensor(zp, m, 0.84, z2, Op.mult, Op.add)
                nc.scalar.activation(da, zp, Act.Identity, bias=cprime_b, scale=0.0375)

                # dv = (v-1)*(-c po - k) - k
                # q's: contiguous fp32 tensor_scalar hits the DVE 2x mode;
                # the (v-1)*q products go to the otherwise-idle GPSIMD
                nc.vector.tensor_scalar(qm, po, -1.2, -0.8, Op.mult, Op.add)
                nc.vector.tensor_scalar(qg, po, -0.7, -0.5, Op.mult, Op.add)
                nc.vector.tensor_scalar(qt, po, -0.3, -0.4, Op.mult, Op.add)
                nc.gpsimd.tensor_scalar(m1, m, 1.0, None, Op.subtract)
                nc.gpsimd.tensor_scalar(g1, g, 1.0, None, Op.subtract)
                nc.gpsimd.tensor_scalar(t1, t_, 1.0, None, Op.subtract)
                nc.gpsimd.tensor_tensor(pm, m1, qm, Op.mult)
                nc.gpsimd.tensor_tensor(pg, g1, qg, Op.mult)
                nc.gpsimd.tensor_tensor(pt, t1, qt, Op.mult)
                nc.scalar.activation(dm, pm, Act.Identity, bias=km_b, scale=1.0)
                nc.scalar.activation(dg, pg, Act.Identity, bias=kg_b, scale=1.0)
                nc.scalar.activation(dt, pt, Act.Identity, bias=kt_b, scale=1.0)

                # out-DMA issued from the ACT queue: it directly follows the
                # final D writes on the same engine, so its wait is already
                # satisfied and it never blocks the sync queue's in-DMAs
                nc.scalar.dma_start(o_d[:, f0 : f0 + fw, :], D[:, :fw, :])

    nc.compile()
    return nc


def _get_nc():
    key = (_Q, _F)
    if key not in _CACHE:
        _CACHE[key] = _build_nc(_Q, _F)
    return _CACHE[key]


def _run_device(x, force, trace=False, tmpdir=None):
    """Shard x [8M,5] over 8 cores, run the Bass kernel, gather drift."""
    from concourse.bass_utils import run_bass_kernel_spmd

    nc = _get_nc()

    consts_np = np.empty((_P, 5), dtype=np.float32)
    consts_np[:, 0] = force
    consts_np[:, 1] = -0.035
    consts_np[:, 2] = -0.8
    consts_np[:, 3] = -0.5
    consts_np[:, 4] = -0.4

    in_maps = []
    for i in range(_NCORES):
        shard = np.zeros((_P * _Q, 5), dtype=np.float32)
        shard[:_RPC] = x[i * _RPC : (i + 1) * _RPC]
        in_maps.append({"x": shard.reshape(_P, _Q, 5), "consts": consts_np})

    res = run_bass_kernel_spmd(
        nc, in_maps, list(range(_NCORES)), trace=trace, tmpdir=tmpdir
    )

    drift = np.empty((_B, 5), dtype=np.float32)
    for i in range(_NCORES):
        drift[i * _RPC : (i + 1) * _RPC] = res.results[i]["drift"].reshape(
            _P * _Q, 5
        )[:_RPC]
    return drift, res


def kernel(t, x):
    t = np.asarray(t, dtype=np.float32)
    x = np.ascontiguousarray(np.asarray(x, dtype=np.float32))
    force = np.float32(0.5 * np.sin(6.283185307179586 * float(t[0]) + 0.0))
    drift, _ = _run_device(x, force, trace=False)
    diffusion = np.broadcast_to(_DSIG, x.shape)
    return drift, diffusion


# revision 21
# speedup vs baseline: 3.3485x; 3.3485x over previous
"""Trainium2 Bass/Tile kernel for the HairBundle SDE drift+diffusion.

Contract: kernel(t, x) takes the FULL inputs (t: [1] f32, x: [8_000_000, 5]
f32) and returns the full (drift, diffusion) pair, matching reference().

Strategy
--------
Trivially data-parallel over the sample-path axis: 8 NeuronCores, each core
takes 1M rows, padded to 128*7813 rows and laid out [128 partitions, 7813
rows/partition, 5 components] so every DMA is dense (contiguous 5*F floats
per partition).  Per 128xFx5 tile the drift is evaluated with 9 VectorE
streams (tensor_tensor / scalar_tensor_tensor) + 9 ScalarE(ACT) streams
(sigmoid + affine Identity ops), reading/writing the interleaved component
planes through stride-5 access patterns.  The diffusion output is a
constant broadcast and is produced host-side for free.

Math (constants folded from the reference):
    d  = h - a;  po = sigmoid(4 d)
    dh = 0.375*(2*(-1.8 h + a) + po) + force         (ACT bias = force)
    da = -0.06*((5/3) dh + (h + 0.75 a - 0.525 m)) + (0.1*force - 0.035)
    dv = (v - 1)*(-c*po - k) - k   for (v,c,k) in
         (m,1.2,0.8), (g,0.7,0.5), (t,0.3,0.4)
    force = 0.5*sin(2*pi*t)
"""

import numpy as np

_B = 8_000_000
_NCORES = 8
_RPC = _B // _NCORES            # rows per core = 1_000_000
_P = 128
_Q = -(-_RPC // _P)             # 7813 rows per partition (padded)
_PADROWS = _P * _Q - _RPC       # 64 junk rows at the tail of each shard
_F = 1024                       # rows-per-partition per SBUF tile
_DSIG = np.array([0.05, 0.02, 0.0, 0.0, 0.0], dtype=np.float32)

_CACHE = {}


def _build_nc(q, f):
    """Build the per-core Bass program for [128, q, 5] in/out, tile width f."""
    import concourse.bacc as bacc
    import concourse.mybir as mybir
    import concourse.tile as tile

    f32 = mybir.dt.float32
    Act = mybir.ActivationFunctionType
    Op = mybir.AluOpType

    nc = bacc.Bacc("TRN2", debug=False)
    x_d = nc.dram_tensor("x", [_P, q, 5], f32, kind="ExternalInput").ap()
    c_d = nc.dram_tensor("consts", [_P, 5], f32, kind="ExternalInput").ap()
    o_d = nc.dram_tensor("drift", [_P, q, 5], f32, kind="ExternalOutput").ap()

    ntiles = -(-q // f)

    with tile.TileContext(nc) as tc:
        with (
            tc.tile_pool(name="io", bufs=3) as io_pool,
            tc.tile_pool(name="tmp", bufs=2) as tmp_pool,
            tc.tile_pool(name="cst", bufs=1) as cst_pool,
        ):
            consts = cst_pool.tile([_P, 5], f32, name="consts_sb")
            nc.sync.dma_start(consts[:, :], c_d[:, :])
            force_b = consts[:, 0:1]
            cprime_b = consts[:, 1:2]
            km_b = consts[:, 2:3]   # -0.8
            kg_b = consts[:, 3:4]   # -0.5
            kt_b = consts[:, 4:5]   # -0.4

            for ti in range(ntiles):
                f0 = ti * f
                fw = min(f, q - f0)

                X = io_pool.tile([_P, f, 5], f32, tag="X", name="X", bufs=3)
                nc.sync.dma_start(X[:, :fw, :], x_d[:, f0 : f0 + fw, :])
                D = io_pool.tile([_P, f, 5], f32, tag="D", name="D", bufs=2)

                h = X[:, :fw, 0]
                a = X[:, :fw, 1]
                m = X[:, :fw, 2]
                g = X[:, :fw, 3]
                t_ = X[:, :fw, 4]
                dh = D[:, :fw, 0]
                da = D[:, :fw, 1]
                dm = D[:, :fw, 2]
                dg = D[:, :fw, 3]
                dt = D[:, :fw, 4]

                def T(nm, bufs=2):
                    # bufs=1 for temps consumed on the same engine that wrote
                    # them (in-order engines make the WAR free); bufs=2 for
                    # cross-engine temps so tiles can pipeline
                    return tmp_pool.tile([_P, f], f32, tag=nm, name=nm, bufs=bufs)[
                        :, :fw
                    ]

                d = T("d")
                po = T("po")
                u1 = T("u1")
                u2 = T("u2")
                zA = T("zA", 1)
                z2 = T("z2", 1)
                zp = T("zp")
                qm = T("qm")
                qg = T("qg")
                qt = T("qt")

                # d = h - a ; po = sigmoid(4 d)
                nc.vector.tensor_tensor(d, h, a, Op.subtract)
                nc.scalar.activation(po, d, Act.Sigmoid, scale=4.0)

                # dh = 0.375*(2*(a - 1.8 h) + po) + force
                nc.vector.scalar_tensor_tensor(u1, h, -1.8, a, Op.mult, Op.add)
                nc.vector.scalar_tensor_tensor(u2, u1, 2.0, po, Op.mult, Op.add)
                nc.scalar.activation(dh, u2, Act.Identity, bias=force_b, scale=0.375)

                # da = 0.0375*(2h - 3.2a - po + 0.84m) - 0.035
                # 2h - 3.2a = 4.7 d + 1.5 u1  (keeps these STTs contiguous)
                nc.vector.scalar_tensor_tensor(zA, d, 4.7 / 1.5, u1, Op.mult, Op.add)
                nc.vector.scalar_tensor_tensor(z2, zA, 1.5, po, Op.mult, Op.subtract)
                nc.vector.scalar_tensor_tensor(zp, m, 0.84, z2, Op.mult, Op.add)
                nc.scalar.activation(da, zp, Act.Identity, bias=cprime_b, scale=0.0375)

                # dv + k = (v-1)*(-c po - k); the STT writes D directly and the
                # host subtracts k per channel during the gather (same fp32
                # rounding as doing it here, but saves 3 strided ACT passes)
                nc.scalar.activation(qm, po, Act.Identity, bias=km_b, scale=-1.2)
                nc.scalar.activation(qg, po, Act.Identity, bias=kg_b, scale=-0.7)
                nc.scalar.activation(qt, po, Act.Identity, bias=kt_b, scale=-0.3)
                nc.vector.scalar_tensor_tensor(dm, m, 1.0, qm, Op.subtract, Op.mult)
                nc.vector.scalar_tensor_tensor(dg, g, 1.0, qg, Op.subtract, Op.mult)
                nc.vector.scalar_tensor_tensor(dt, t_, 1.0, qt, Op.subtract, Op.mult)

                # out-DMA on the (otherwise idle) gpsimd SWDGE queue so its
                # wait-on-compute doesn't block any compute engine's stream
                nc.gpsimd.dma_start(o_d[:, f0 : f0 + fw, :], D[:, :fw, :])

    nc.compile()
    return nc


def _get_nc():
    key = (_Q, _F)
    if key not in _CACHE:
        _CACHE[key] = _build_nc(_Q, _F)
    return _CACHE[key]


def _run_device(x, force, trace=False, tmpdir=None):
    """Shard x [8M,5] over 8 cores, run the Bass kernel, gather drift."""
    from concourse.bass_utils import run_bass_kernel_spmd

    nc = _get_nc()

    consts_np = np.empty((_P, 5), dtype=np.float32)
    consts_np[:, 0] = force
    consts_np[:, 1] = -0.035
    consts_np[:, 2] = -0.8
    consts_np[:, 3] = -0.5
    consts_np[:, 4] = -0.4

    in_maps = []
    for i in range(_NCORES):
        shard = np.zeros((_P * _Q, 5), dtype=np.float32)
        shard[:_RPC] = x[i * _RPC : (i + 1) * _RPC]
        in_maps.append({"x": shard.reshape(_P, _Q, 5), "consts": consts_np})

    res = run_bass_kernel_spmd(
        nc, in_maps, list(range(_NCORES)), trace=trace, tmpdir=tmpdir
    )

    drift = np.empty((_B, 5), dtype=np.float32)
    for i in range(_NCORES):
        drift[i * _RPC : (i + 1) * _RPC] = res.results[i]["drift"].reshape(
            _P * _Q, 5
        )[:_RPC]
    # device leaves channels 2..4 k-shifted by (0.8, 0.5, 0.4)
    drift[:, 2] -= np.float32(0.8)
    drift[:, 3] -= np.float32(0.5)
    drift[:, 4] -= np.float32(0.4)
    return drift, res


def kernel(t, x):
    t = np.asarray(t, dtype=np.float32)
    x = np.ascontiguousarray(np.asarray(x, dtype=np.float32))
    force = np.float32(0.5 * np.sin(6.283185307179586 * float(t[0]) + 0.0))
    drift, _ = _run_device(x, force, trace=False)
    diffusion = np.broadcast_to(_DSIG, x.shape)
    return drift, diffusion


# revision 26
# speedup vs baseline: 3.4096x; 1.0182x over previous
"""Trainium2 Bass/Tile kernel for the HairBundle SDE drift+diffusion.

Contract: kernel(t, x) takes the FULL inputs (t: [1] f32, x: [8_000_000, 5]
f32) and returns the full (drift, diffusion) pair, matching reference().

Strategy
--------
Trivially data-parallel over the sample-path axis: 8 NeuronCores, each core
takes 1M rows, padded to 128*7813 rows and laid out [128 partitions, 7813
rows/partition, 5 components] so every DMA is dense (contiguous 5*F floats
per partition).  Per 128xFx5 tile the drift is evaluated with 9 VectorE
streams (tensor_tensor / scalar_tensor_tensor) + 9 ScalarE(ACT) streams
(sigmoid + affine Identity ops), reading/writing the interleaved component
planes through stride-5 access patterns.  The diffusion output is a
constant broadcast and is produced host-side for free.

Math (constants folded from the reference):
    d  = h - a;  po = sigmoid(4 d)
    dh = 0.375*(2*(-1.8 h + a) + po) + force         (ACT bias = force)
    da = -0.06*((5/3) dh + (h + 0.75 a - 0.525 m)) + (0.1*force - 0.035)
    dv = (v - 1)*(-c*po - k) - k   for (v,c,k) in
         (m,1.2,0.8), (g,0.7,0.5), (t,0.3,0.4)
    force = 0.5*sin(2*pi*t)
"""

import numpy as np

_B = 8_000_000
_NCORES = 8
_RPC = _B // _NCORES            # rows per core = 1_000_000
_P = 128
_Q = -(-_RPC // _P)             # 7813 rows per partition (padded)
_PADROWS = _P * _Q - _RPC       # 64 junk rows at the tail of each shard
_F = 1024                       # rows-per-partition per SBUF tile
_DSIG = np.array([0.05, 0.02, 0.0, 0.0, 0.0], dtype=np.float32)

_CACHE = {}


def _build_nc(q, f):
    """Build the per-core Bass program for [128, q, 5] in/out, tile width f."""
    import concourse.bacc as bacc
    import concourse.mybir as mybir
    import concourse.tile as tile

    f32 = mybir.dt.float32
    Act = mybir.ActivationFunctionType
    Op = mybir.AluOpType

    nc = bacc.Bacc("TRN2", debug=False)
    x_d = nc.dram_tensor("x", [_P, q, 5], f32, kind="ExternalInput").ap()
    c_d = nc.dram_tensor("consts", [_P, 6], f32, kind="ExternalInput").ap()
    o_d = nc.dram_tensor("drift", [_P, q, 5], f32, kind="ExternalOutput").ap()

    ntiles = -(-q // f)

    with tile.TileContext(nc) as tc:
        with (
            tc.tile_pool(name="io", bufs=3) as io_pool,
            tc.tile_pool(name="tmp", bufs=2) as tmp_pool,
            tc.tile_pool(name="cst", bufs=1) as cst_pool,
        ):
            consts = cst_pool.tile([_P, 6], f32, name="consts_sb")
            nc.sync.dma_start(consts[:, :], c_d[:, :])
            force_b = consts[:, 0:1]
            cprime_b = consts[:, 1:2]
            km_b = consts[:, 2:3]   # -0.8
            kg_b = consts[:, 3:4]   # -0.5
            kt_b = consts[:, 4:5]   # -0.4
            neg1_b = consts[:, 5:6]  # -1.0

            for ti in range(ntiles):
                f0 = ti * f
                fw = min(f, q - f0)

                X = io_pool.tile([_P, f, 5], f32, tag="X", name="X", bufs=3)
                nc.sync.dma_start(X[:, :fw, :], x_d[:, f0 : f0 + fw, :])
                D = io_pool.tile([_P, f, 5], f32, tag="D", name="D", bufs=2)

                h = X[:, :fw, 0]
                a = X[:, :fw, 1]
                m = X[:, :fw, 2]
                g = X[:, :fw, 3]
                t_ = X[:, :fw, 4]
                dh = D[:, :fw, 0]
                da = D[:, :fw, 1]
                dm = D[:, :fw, 2]
                dg = D[:, :fw, 3]
                dt = D[:, :fw, 4]

                def T(nm, bufs=2):
                    # bufs=1 for temps consumed on the same engine that wrote
                    # them (in-order engines make the WAR free); bufs=2 for
                    # cross-engine temps so tiles can pipeline
                    return tmp_pool.tile([_P, f], f32, tag=nm, name=nm, bufs=bufs)[
                        :, :fw
                    ]

                d = T("d")
                po = T("po")
                u1 = T("u1")
                u2 = T("u2")
                zA = T("zA", 1)
                z2 = T("z2", 1)
                zp = T("zp")
                qm = T("qm")
                qg = T("qg")
                qt = T("qt")
                v1m = T("v1m")

                # d = h - a ; po = sigmoid(4 d)
                nc.vector.tensor_tensor(d, h, a, Op.subtract)
                nc.scalar.activation(po, d, Act.Sigmoid, scale=4.0)

                # dh = 0.375*(2*(a - 1.8 h) + po) + force
                nc.vector.scalar_tensor_tensor(u1, h, -1.8, a, Op.mult, Op.add)
                nc.vector.scalar_tensor_tensor(u2, u1, 2.0, po, Op.mult, Op.add)
                nc.scalar.activation(dh, u2, Act.Identity, bias=force_b, scale=0.375)

                # v1m = m - 1, contiguous: reused by both da's m-term and dm
                nc.scalar.activation(v1m, m, Act.Identity, bias=neg1_b, scale=1.0)

                # da = 0.0375*(2h - 3.2a - po + 0.84(m-1)) - 0.0035
                # 2h - 3.2a = 4.7 d + 1.5 u1  (keeps these STTs contiguous)
                nc.vector.scalar_tensor_tensor(zA, d, 4.7 / 1.5, u1, Op.mult, Op.add)
                nc.vector.scalar_tensor_tensor(z2, zA, 1.5, po, Op.mult, Op.subtract)
                nc.vector.scalar_tensor_tensor(zp, v1m, 0.84, z2, Op.mult, Op.add)
                nc.scalar.activation(da, zp, Act.Identity, bias=cprime_b, scale=0.0375)

                # dv + k = (v-1)*(-c po - k); the STT writes D directly and the
                # host subtracts k per channel during the gather (same fp32
                # rounding as doing it here, but saves 3 strided ACT passes)
                nc.scalar.activation(qm, po, Act.Identity, bias=km_b, scale=-1.2)
                nc.scalar.activation(qg, po, Act.Identity, bias=kg_b, scale=-0.7)
                nc.scalar.activation(qt, po, Act.Identity, bias=kt_b, scale=-0.3)
                nc.vector.tensor_tensor(dm, v1m, qm, Op.mult)
                nc.vector.scalar_tensor_tensor(dg, g, 1.0, qg, Op.subtract, Op.mult)
                nc.vector.scalar_tensor_tensor(dt, t_, 1.0, qt, Op.subtract, Op.mult)

                # out-DMA on the (otherwise idle) gpsimd SWDGE queue so its
                # wait-on-compute doesn't block any compute engine's stream
                nc.gpsimd.dma_start(o_d[:, f0 : f0 + fw, :], D[:, :fw, :])

    nc.compile()
    return nc


def _get_nc():
    key = (_Q, _F)
    if key not in _CACHE:
        _CACHE[key] = _build_nc(_Q, _F)
    return _CACHE[key]


def _run_device(x, force, trace=False, tmpdir=None):
    """Shard x [8M,5] over 8 cores, run the Bass kernel, gather drift."""
    from concourse.bass_utils import run_bass_kernel_spmd

    nc = _get_nc()

    consts_np = np.empty((_P, 6), dtype=np.float32)
    consts_np[:, 0] = force
    consts_np[:, 1] = -0.0035
    consts_np[:, 5] = -1.0
    consts_np[:, 2] = -0.8
    consts_np[:, 3] = -0.5
    consts_np[:, 4] = -0.4

    in_maps = []
    for i in range(_NCORES):
        shard = np.zeros((_P * _Q, 5), dtype=np.float32)
        shard[:_RPC] = x[i * _RPC : (i + 1) * _RPC]
        in_maps.append({"x": shard.reshape(_P, _Q, 5), "consts": consts_np})

    res = run_bass_kernel_spmd(
        nc, in_maps, list(range(_NCORES)), trace=trace, tmpdir=tmpdir
    )

    drift = np.empty((_B, 5), dtype=np.float32)
    for i in range(_NCORES):
        drift[i * _RPC : (i + 1) * _RPC] = res.results[i]["drift"].reshape(
            _P * _Q, 5
        )[:_RPC]
    # device leaves channels 2..4 k-shifted by (0.8, 0.5, 0.4)
    drift[:, 2] -= np.float32(0.8)
    drift[:, 3] -= np.float32(0.5)
    drift[:, 4] -= np.float32(0.4)
    return drift, res


def kernel(t, x):
    t = np.asarray(t, dtype=np.float32)
    x = np.ascontiguousarray(np.asarray(x, dtype=np.float32))
    force = np.float32(0.5 * np.sin(6.283185307179586 * float(t[0]) + 0.0))
    drift, _ = _run_device(x, force, trace=False)
    diffusion = np.broadcast_to(_DSIG, x.shape)
    return drift, diffusion


# revision 27
# speedup vs baseline: 3.6573x; 1.0727x over previous
"""Trainium2 Bass/Tile kernel for the HairBundle SDE drift+diffusion.

Contract: kernel(t, x) takes the FULL inputs (t: [1] f32, x: [8_000_000, 5]
f32) and returns the full (drift, diffusion) pair, matching reference().

Strategy
--------
Trivially data-parallel over the sample-path axis: 8 NeuronCores, each core
takes 1M rows padded to 128*7813.  The host hands each core PLANAR data
[128 partitions, 5 components, 7813 rows] (one numpy transpose each way) so
that every DMA is dense AND every on-chip access pattern is unit-stride --
strided (interleaved) operands run at half rate on both VectorE and
ScalarE, so de-interleaving on the host removes the whole tax.  Per tile
the drift is 9 contiguous VectorE streams + 6 ScalarE streams; loads issue
from the sync queue, stores from the gpsimd queue so neither blocks.
The diffusion output is a constant broadcast, produced host-side for free.

Math (constants folded from the reference):
    d  = h - a;  po = sigmoid(4 d)
    dh = 0.375*(2*(a - 1.8 h) + po) + force          (ACT bias = force)
    da = 0.0375*(2h - 3.2a - po + 0.84 m) - 0.035
    dv + k = (v - 1)*(-c*po - k)   for (v,c,k) in
         (m,1.2,0.8), (g,0.7,0.5), (t,0.3,0.4)   [host subtracts k]
    force = 0.5*sin(2*pi*t)
"""

import numpy as np

_B = 8_000_000
_NCORES = 8
_RPC = _B // _NCORES            # rows per core = 1_000_000
_P = 128
_Q = -(-_RPC // _P)             # 7813 rows per partition (padded by 64 rows)
_F = 1024                       # rows-per-partition per SBUF tile
_DSIG = np.array([0.05, 0.02, 0.0, 0.0, 0.0], dtype=np.float32)

_CACHE = {}


def _build_nc(q, f):
    """Per-core Bass program: x [128, 5, q] planar -> drift [128, 5, q]."""
    import concourse.bacc as bacc
    import concourse.mybir as mybir
    import concourse.tile as tile

    f32 = mybir.dt.float32
    Act = mybir.ActivationFunctionType
    Op = mybir.AluOpType

    nc = bacc.Bacc("TRN2", debug=False)
    x_d = nc.dram_tensor("x", [_P, 5, q], f32, kind="ExternalInput").ap()
    c_d = nc.dram_tensor("consts", [_P, 5], f32, kind="ExternalInput").ap()
    o_d = nc.dram_tensor("drift", [_P, 5, q], f32, kind="ExternalOutput").ap()

    ntiles = -(-q // f)

    with tile.TileContext(nc) as tc:
        with (
            tc.tile_pool(name="io", bufs=3) as io_pool,
            tc.tile_pool(name="tmp", bufs=2) as tmp_pool,
            tc.tile_pool(name="cst", bufs=1) as cst_pool,
        ):
            consts = cst_pool.tile([_P, 5], f32, name="consts_sb")
            nc.sync.dma_start(consts[:, :], c_d[:, :])
            force_b = consts[:, 0:1]
            cprime_b = consts[:, 1:2]
            km_b = consts[:, 2:3]   # -0.8
            kg_b = consts[:, 3:4]   # -0.5
            kt_b = consts[:, 4:5]   # -0.4

            for ti in range(ntiles):
                f0 = ti * f
                fw = min(f, q - f0)

                X = io_pool.tile([_P, 5, f], f32, tag="X", name="X", bufs=3)
                nc.sync.dma_start(X[:, :, :fw], x_d[:, :, f0 : f0 + fw])
                D = io_pool.tile([_P, 5, f], f32, tag="D", name="D", bufs=2)

                h = X[:, 0, :fw]
                a = X[:, 1, :fw]
                m = X[:, 2, :fw]
                g = X[:, 3, :fw]
                t_ = X[:, 4, :fw]
                dh = D[:, 0, :fw]
                da = D[:, 1, :fw]
                dm = D[:, 2, :fw]
                dg = D[:, 3, :fw]
                dt = D[:, 4, :fw]

                def T(nm, bufs=2):
                    # bufs=1 for temps consumed on the same engine that wrote
                    # them (in-order engines make the WAR free)
                    return tmp_pool.tile([_P, f], f32, tag=nm, name=nm, bufs=bufs)[
                        :, :fw
                    ]

                d = T("d")
                po = T("po")
                u1 = T("u1")
                u2 = T("u2")
                zA = T("zA", 1)
                z2 = T("z2", 1)
                zp = T("zp")
                qm = T("qm")
                qg = T("qg")
                qt = T("qt")

                # d = h - a ; po = sigmoid(4 d)
                nc.vector.tensor_tensor(d, h, a, Op.subtract)
                nc.scalar.activation(po, d, Act.Sigmoid, scale=4.0)

                # dh = 0.375*(2*(a - 1.8 h) + po) + force
                nc.vector.scalar_tensor_tensor(u1, h, -1.8, a, Op.mult, Op.add)
                nc.vector.scalar_tensor_tensor(u2, u1, 2.0, po, Op.mult, Op.add)
                nc.scalar.activation(dh, u2, Act.Identity, bias=force_b, scale=0.375)

                # da = 0.0375*(2h - 3.2a - po + 0.84 m) - 0.035
                # 2h - 3.2a = 4.7 d + 1.5 u1
                nc.vector.scalar_tensor_tensor(zA, d, 4.7 / 1.5, u1, Op.mult, Op.add)
                nc.vector.scalar_tensor_tensor(z2, zA, 1.5, po, Op.mult, Op.subtract)
                nc.vector.scalar_tensor_tensor(zp, m, 0.84, z2, Op.mult, Op.add)
                nc.scalar.activation(da, zp, Act.Identity, bias=cprime_b, scale=0.0375)

                # dv + k = (v-1)*(-c po - k); host subtracts k after gather
                nc.scalar.activation(qm, po, Act.Identity, bias=km_b, scale=-1.2)
                nc.scalar.activation(qg, po, Act.Identity, bias=kg_b, scale=-0.7)
                nc.scalar.activation(qt, po, Act.Identity, bias=kt_b, scale=-0.3)
                nc.vector.scalar_tensor_tensor(dm, m, 1.0, qm, Op.subtract, Op.mult)
                nc.vector.scalar_tensor_tensor(dg, g, 1.0, qg, Op.subtract, Op.mult)
                nc.vector.scalar_tensor_tensor(dt, t_, 1.0, qt, Op.subtract, Op.mult)

                # out-DMA on the (otherwise idle) gpsimd SWDGE queue so its
                # wait-on-compute doesn't block the sync queue's in-DMAs
                nc.gpsimd.dma_start(o_d[:, :, f0 : f0 + fw], D[:, :, :fw])

    nc.compile()
    return nc


def _get_nc():
    key = (_Q, _F)
    if key not in _CACHE:
        _CACHE[key] = _build_nc(_Q, _F)
    return _CACHE[key]


def _run_device(x, force, trace=False, tmpdir=None):
    """Shard x [8M,5] over 8 cores (planar per-core layout), gather drift."""
    from concourse.bass_utils import run_bass_kernel_spmd

    nc = _get_nc()

    consts_np = np.empty((_P, 5), dtype=np.float32)
    consts_np[:, 0] = force
    consts_np[:, 1] = -0.035
    consts_np[:, 2] = -0.8
    consts_np[:, 3] = -0.5
    consts_np[:, 4] = -0.4

    in_maps = []
    for i in range(_NCORES):
        shard = np.zeros((_P, _Q, 5), dtype=np.float32)
        shard.reshape(_P * _Q, 5)[:_RPC] = x[i * _RPC : (i + 1) * _RPC]
        planar = np.ascontiguousarray(shard.transpose(0, 2, 1))  # [P, 5, Q]
        in_maps.append({"x": planar, "consts": consts_np})

    res = run_bass_kernel_spmd(
        nc, in_maps, list(range(_NCORES)), trace=trace, tmpdir=tmpdir
    )

    drift = np.empty((_B, 5), dtype=np.float32)
    for i in range(_NCORES):
        out = res.results[i]["drift"]  # [P, 5, Q] planar
        rows = out.transpose(0, 2, 1).reshape(_P * _Q, 5)
        drift[i * _RPC : (i + 1) * _RPC] = rows[:_RPC]
    # device leaves channels 2..4 k-shifted by (0.8, 0.5, 0.4)
    drift[:, 2] -= np.float32(0.8)
    drift[:, 3] -= np.float32(0.5)
    drift[:, 4] -= np.float32(0.4)
    return drift, res


def kernel(t, x):
    t = np.asarray(t, dtype=np.float32)
    x = np.asarray(x, dtype=np.float32)
    force = np.float32(0.5 * np.sin(6.283185307179586 * float(t[0]) + 0.0))
    drift, _ = _run_device(x, force, trace=False)
    diffusion = np.broadcast_to(_DSIG, x.shape)
    return drift, diffusion
